# revision 29
# baseline (speedup 1.0000x reference)
"""Trainium2 Bass kernel for batched multi-head attention.

Problem: B=8, L=1024, D=1024, H=16 heads (DH=64), fp32 reference:
    q/k/v = x @ W{q,k,v}.T ; scores = q k^T per head, masked (0/1 mask,
    -1e9 fill), softmax(scale=1/8), out = (weights @ v) @ Wo.T

Distribution: pure data parallel — batch b -> NeuronCore b (B == n_cores == 8).
No collectives needed; each core runs the full attention for its batch.

Per-core algorithm (layouts chosen so no on-device transposes are needed;
the host pre-transposes x, weights and mask instead):
  - inputs per core: xT=[d,l] fp16, WqT/WkT=[d,e] fp16, WvT=[d,e] bf16,
    WoT=[e,f] fp16, maskT=[m,l] bf16 (m=key idx, l=query idx)
  - Kt[e,l] = WkT.T @ xT  (PE fp16, fp32 PSUM accum over d), Qt likewise,
    V[m,e] = xT.T @ WvT stored per m-tile as [128, 16 heads, 65] bf16 with a
    constant-1 column per head ("Vones"): the AV matmul then produces the
    softmax denominator for free in its row 64.
  - per head pair (heads 2*et, 2*et+1 share the PE array via 64-row tiles):
      S^T[m,l] = Kt_h.T @ Qt_h           (PE, K=64, rows j*64..j*64+63)
      E = exp(S^T/8 - 32)                (ACT; constant shift keeps exp in
                                          fp32/bf16 range, cancels in softmax)
      P = E * maskT                      (DVE, bf16 2x mode)
      AVpsum[65,l] += Vones.T @ P        (PE, K=128; trails the scores by 2
                                          m-tiles so PE never stalls on ACT)
      rz = 1/Z (approx recip, fp32->bf16), Rb = ones64.T @ rz (K=1 bcast MM)
      OT[e,l] = AVpsum[0:64] * Rb        (DVE -> fp16 OT buffer)
  - Q projection for the NEXT head pair is emitted mid-loop as PE filler.
  - y[l,f] = OT.T @ WoT (PE fp16) -> f16 out, host casts to f32.
"""

import sys

if "/opt/trn_rl_repo" not in sys.path:
    sys.path.insert(0, "/opt/trn_rl_repo")

import numpy as np
import ml_dtypes

B = 8
L = 1024
D = 1024
H = 16
DH = 64
NT = 8
SCALE = 0.125
EXP_SHIFT = -32.0
TRAIL = 2  # AV matmuls trail the score matmuls by this many m-tiles
N_CORES = 8

_NC_CACHE = None


def _build():
    import concourse.bass as bass  # noqa: F401
    from concourse import bacc, mybir
    import concourse.tile as tile

    f32 = mybir.dt.float32
    f16 = mybir.dt.float16
    bf16 = mybir.dt.bfloat16
    Exp = mybir.ActivationFunctionType.Exp

    nc = bacc.Bacc(None, target_bir_lowering=False)

    xT = nc.declare_dram_parameter("xT", [D, L], f16, isOutput=False)
    wqT = nc.declare_dram_parameter("wqT", [D, D], f16, isOutput=False)
    wkT = nc.declare_dram_parameter("wkT", [D, D], f16, isOutput=False)
    wvT = nc.declare_dram_parameter("wvT", [D, D], bf16, isOutput=False)
    woT = nc.declare_dram_parameter("woT", [D, D], f16, isOutput=False)
    mskT = nc.declare_dram_parameter("maskT", [L, L], bf16, isOutput=False)
    out = nc.declare_dram_parameter("out", [L, D], f16, isOutput=True)

    with tile.TileContext(nc) as tc:
        with (
            tc.tile_pool(name="big", bufs=1) as big,
            tc.tile_pool(name="pb", bufs=TRAIL + 1) as pbp,
            tc.tile_pool(name="sm", bufs=2) as smp,
            tc.tile_pool(name="rb", bufs=2) as rbp,
            tc.tile_pool(name="avr", bufs=2) as avrp,
            tc.tile_pool(name="yb", bufs=2) as ybp,
            tc.tile_pool(name="psmm", bufs=2, space="PSUM") as psmm,
            tc.tile_pool(name="psav", bufs=2, space="PSUM") as psav,
        ):
            x_t = [big.tile([128, L], f16, tag=f"x{i}", name=f"x{i}") for i in range(NT)]
            wq_t = [big.tile([128, D], f16, tag=f"wq{i}", name=f"wq{i}") for i in range(NT)]
            wk_t = [big.tile([128, D], f16, tag=f"wk{i}", name=f"wk{i}") for i in range(NT)]
            wv_t = [big.tile([128, D], bf16, tag=f"wv{i}", name=f"wv{i}") for i in range(NT)]
            wo_t = [big.tile([128, D], f16, tag=f"wo{i}", name=f"wo{i}") for i in range(NT)]
            mk_t = [big.tile([128, L], bf16, tag=f"mk{i}", name=f"mk{i}") for i in range(NT)]
            qt_t = [big.tile([128, L], f16, tag=f"qt{i}", name=f"qt{i}") for i in range(NT)]
            kt_t = [big.tile([128, L], f16, tag=f"kt{i}", name=f"kt{i}") for i in range(NT)]
            v_t = [big.tile([128, H, 65], bf16, tag=f"v{i}", name=f"v{i}") for i in range(NT)]
            ot_t = [big.tile([128, L], f16, tag=f"ot{i}", name=f"ot{i}") for i in range(NT)]
            ones64 = big.tile([33, 64], bf16, tag="ones64")
            negC = big.tile([128, 1], f32, tag="negC")

            # input DMAs (K/V/x first: K and V projections run first)
            for i in range(NT):
                sl = slice(i * 128, (i + 1) * 128)
                nc.sync.dma_start(out=wk_t[i][:, :], in_=wkT[sl, :])
                nc.sync.dma_start(out=x_t[i][:, :], in_=xT[sl, :])
            for i in range(NT):
                sl = slice(i * 128, (i + 1) * 128)
                nc.sync.dma_start(out=wv_t[i][:, :], in_=wvT[sl, :])
                nc.sync.dma_start(out=wq_t[i][:, :], in_=wqT[sl, :])
            for i in range(NT):
                sl = slice(i * 128, (i + 1) * 128)
                nc.sync.dma_start(out=mk_t[i][:, :], in_=mskT[sl, :])
                nc.sync.dma_start(out=wo_t[i][:, :], in_=woT[sl, :])

            nc.vector.memset(ones64[:, :], 1.0)
            nc.vector.memset(negC[:, :], EXP_SHIFT)
            for i in range(NT):
                nc.vector.memset(v_t[i][:, :, 64:65], 1.0)

            def qk_proj(w_tiles, dst, et, use_av=False):
                if use_av:
                    ps = psav.tile([128, L], f32, tag="av", name=f"projps{et}")
                else:
                    ps = psmm.tile([128, L], f32, tag="mm", name=f"projps{et}")
                for c in range(2):
                    cs = slice(c * 512, (c + 1) * 512)
                    for dt in range(NT):
                        nc.tensor.matmul(
                            ps[:, cs],
                            lhsT=w_tiles[dt][:, et * 128 : (et + 1) * 128],
                            rhs=x_t[dt][:, cs],
                            start=(dt == 0),
                            stop=(dt == NT - 1),
                        )
                nc.vector.tensor_copy(out=dst[et][:, :], in_=ps[:, :])

            def v_proj(mt):
                ps = psav.tile([128, L], f32, tag="av", name=f"vps{mt}")
                for c in range(2):
                    cs = slice(c * 512, (c + 1) * 512)
                    for dt in range(NT):
                        nc.tensor.matmul(
                            ps[:, cs],
                            lhsT=x_t[dt][:, mt * 128 : (mt + 1) * 128],
                            rhs=wv_t[dt][:, cs],
                            start=(dt == 0),
                            stop=(dt == NT - 1),
                        )
                nc.vector.tensor_copy(
                    out=v_t[mt][:, :, 0:64],
                    in_=ps[:, :].rearrange("p (h e) -> p h e", h=H),
                )

            # ---- up-front projections: all of K, Q0, then all of V
            for et in range(NT):
                qk_proj(wk_t, kt_t, et)
            qk_proj(wq_t, qt_t, 0)
            for mt in range(NT):
                v_proj(mt)

            # ---- attention: one continuous software pipeline over all
            # (head-pair, m-tile) steps. AV matmuls trail the score matmuls
            # by TRAIL steps globally (across head-pair boundaries) so the PE
            # stream never drains; K/Q projections for the next pair and the
            # deferred normalization flush slot into fixed positions.
            def flush_epilogue(p):
                pet, pavr, przb = p
                psr = psmm.tile([128, L], f32, tag="mm", name="psr")
                for j in range(2):
                    jp = slice(j * 64, (j + 1) * 64)
                    for c in range(2):
                        cs = slice(c * 512, (c + 1) * 512)
                        nc.tensor.matmul(
                            psr[jp, cs],
                            lhsT=ones64[32 * j : 32 * j + 1, :],
                            rhs=przb[32 * j : 32 * j + 1, cs],
                            start=True, stop=True,
                        )
                for j in range(2):
                    jp = slice(j * 64, (j + 1) * 64)
                    rb = rbp.tile([64, L], bf16, name="rb")
                    nc.scalar.copy(out=rb[:, :], in_=psr[jp, :])
                    nc.vector.tensor_mul(
                        ot_t[pet][jp, :], pavr[j][0:64, :], rb[:, :]
                    )

            pending = None
            av = None
            pref = {}
            navr = 0
            for g in range(NT * NT + TRAIL):
                et, mt = divmod(g, NT)
                if g < NT * NT:
                    psj = [
                        psmm.tile([128, L], f32, tag="mm", name=f"sps{j}")
                        for j in range(2)
                    ]
                    for c in range(2):
                        cs = slice(c * 512, (c + 1) * 512)
                        for j in range(2):
                            jp = slice(j * 64, (j + 1) * 64)
                            nc.tensor.matmul(
                                psj[j][:, cs],
                                lhsT=kt_t[et][jp, mt * 128 : (mt + 1) * 128],
                                rhs=qt_t[et][jp, cs],
                                start=True,
                                stop=True,
                            )
                    p2 = pbp.tile([128, 2, L], bf16, name="p2")
                    for j in range(2):
                        nc.scalar.activation(
                            p2[:, j, :], psj[j][:, :], Exp, scale=SCALE, bias=negC[:, :]
                        )
                    nc.vector.tensor_mul(
                        p2[:, :, :],
                        p2[:, :, :],
                        mk_t[mt][:, :].rearrange("p (o l) -> p o l", o=1).to_broadcast([128, 2, L]),
                    )
                    pref[(et, mt)] = p2
                    if mt == 3 and pending is not None:
                        flush_epilogue(pending)
                        pending = None
                    if mt == 5 and et + 1 < NT:
                        qk_proj(wq_t, qt_t, et + 1)
                ag = g - TRAIL
                if ag >= 0:
                    aet, amt = divmod(ag, NT)
                    if amt == 0:
                        av = [
                            psav.tile([65, L], f32, tag="av", name=f"av{j}")
                            for j in range(2)
                        ]
                    p2 = pref.pop((aet, amt))
                    for j in range(2):
                        h = 2 * aet + j
                        for c in range(2):
                            cs = slice(c * 512, (c + 1) * 512)
                            nc.tensor.matmul(
                                av[j][:, cs],
                                lhsT=v_t[amt][:, h, :],
                                rhs=p2[:, j, cs],
                                start=(amt == 0),
                                stop=(amt == NT - 1),
                            )
                    if amt == NT - 1:
                        # single bf16 eviction frees the AV psum immediately;
                        # normalization then runs lazily from SBUF
                        avrs = []
                        for j in range(2):
                            avr = avrp.tile([65, L], bf16, tag="avr", name="avr")
                            nc.vector.tensor_copy(out=avr[:, :], in_=av[j][:, :])
                            avrs.append(avr)
                        zrow = smp.tile([33, L], f32, tag="rz2", name="zrow")
                        nc.scalar.copy(out=zrow[0:1, :], in_=avrs[0][64:65, :])
                        nc.scalar.copy(out=zrow[32:33, :], in_=avrs[1][64:65, :])
                        rz = smp.tile([33, L], f32, tag="rz", name="rz")
                        nc.vector.reciprocal_approx_fast(out=rz[:, :], in_=zrow[:, :])
                        rzb = smp.tile([33, L], bf16, tag="rzb", name="rzb")
                        nc.vector.tensor_copy(out=rzb[:, :], in_=rz[:, :])
                        pending = (aet, avrs, rzb)

            if pending is not None:
                flush_epilogue(pending)
                pending = None

            # ---- output projection: y[l, f] = OT.T @ WoT
            for lt in range(NT):
                pool = psmm if lt % 2 == 0 else psav
                tag = "mm" if lt % 2 == 0 else "av"
                ps = pool.tile([128, L], f32, tag=tag, name="yps")
                for fc in range(2):
                    cs = slice(fc * 512, (fc + 1) * 512)
                    for p in range(NT):
                        nc.tensor.matmul(
                            ps[:, cs],
                            lhsT=ot_t[p][:, lt * 128 : (lt + 1) * 128],
                            rhs=wo_t[p][:, cs],
                            start=(p == 0),
                            stop=(p == NT - 1),
                        )
                y = ybp.tile([128, L], f16, name="y")
                nc.scalar.copy(out=y[:, :], in_=ps[:, :])
                nc.sync.dma_start(out=out[lt * 128 : (lt + 1) * 128, :], in_=y[:, :])

    nc.finalize()
    return nc


def _get_nc():
    global _NC_CACHE
    if _NC_CACHE is None:
        _NC_CACHE = _build()
    return _NC_CACHE


def _make_in_maps(x, mask, Wk, Wv, Wq, Wo):
    f16 = np.float16
    bf16 = ml_dtypes.bfloat16
    wqT = np.ascontiguousarray(Wq.T).astype(f16)
    wkT = np.ascontiguousarray(Wk.T).astype(f16)
    wvT = np.ascontiguousarray(Wv.T).astype(bf16)
    woT = np.ascontiguousarray(Wo.T).astype(f16)
    maskT = np.ascontiguousarray(mask[0].T).astype(bf16)
    in_maps = []
    for b in range(N_CORES):
        in_maps.append(
            {
                "xT": np.ascontiguousarray(x[b].T).astype(f16),
                "wqT": wqT,
                "wkT": wkT,
                "wvT": wvT,
                "woT": woT,
                "maskT": maskT,
            }
        )
    return in_maps


def _run(x, mask, Wk, Wv, Wq, Wo, trace=False):
    from concourse.bass_utils import run_bass_kernel_spmd

    nc = _get_nc()
    in_maps = _make_in_maps(x, mask, Wk, Wv, Wq, Wo)
    res = run_bass_kernel_spmd(nc, in_maps, list(range(N_CORES)), trace=trace)
    y = np.stack([res.results[b]["out"] for b in range(N_CORES)], axis=0)
    return y.astype(np.float32), res


def kernel(x, mask, Wk, Wv, Wq, Wo):
    y, _ = _run(x, mask, Wk, Wv, Wq, Wo, trace=False)
    return y


# revision 30
# speedup vs baseline: 1.0073x; 1.0073x over previous
"""Trainium2 Bass kernel for batched multi-head attention.

Problem: B=8, L=1024, D=1024, H=16 heads (DH=64), fp32 reference:
    q/k/v = x @ W{q,k,v}.T ; scores = q k^T per head, masked (0/1 mask,
    -1e9 fill), softmax(scale=1/8), out = (weights @ v) @ Wo.T

Distribution: pure data parallel — batch b -> NeuronCore b (B == n_cores == 8).
No collectives needed; each core runs the full attention for its batch.

Per-core algorithm (layouts chosen so no on-device transposes are needed;
the host pre-transposes x, weights and mask instead):
  - inputs per core: xT=[d,l] fp16, WqT/WkT=[d,e] fp16, WvT=[d,e] bf16,
    WoT=[e,f] fp16, maskT=[m,l] bf16 (m=key idx, l=query idx)
  - Kt[e,l] = WkT.T @ xT  (PE fp16, fp32 PSUM accum over d), Qt likewise,
    V[m,e] = xT.T @ WvT stored per m-tile as [128, 16 heads, 65] bf16 with a
    constant-1 column per head ("Vones"): the AV matmul then produces the
    softmax denominator for free in its row 64.
  - per head pair (heads 2*et, 2*et+1 share the PE array via 64-row tiles):
      S^T[m,l] = Kt_h.T @ Qt_h           (PE, K=64, rows j*64..j*64+63)
      E = exp(S^T/8 - 32)                (ACT; constant shift keeps exp in
                                          fp32/bf16 range, cancels in softmax)
      P = E * maskT                      (DVE, bf16 2x mode)
      AVpsum[65,l] += Vones.T @ P        (PE, K=128; trails the scores by 2
                                          m-tiles so PE never stalls on ACT)
      rz = 1/Z (approx recip, fp32->bf16), Rb = ones64.T @ rz (K=1 bcast MM)
      OT[e,l] = AVpsum[0:64] * Rb        (DVE -> fp16 OT buffer)
  - Q projection for the NEXT head pair is emitted mid-loop as PE filler.
  - y[l,f] = OT.T @ WoT (PE fp16) -> f16 out, host casts to f32.
"""

import sys

if "/opt/trn_rl_repo" not in sys.path:
    sys.path.insert(0, "/opt/trn_rl_repo")

import numpy as np
import ml_dtypes

B = 8
L = 1024
D = 1024
H = 16
DH = 64
NT = 8
SCALE = 0.125
EXP_SHIFT = -32.0
TRAIL = 2  # AV matmuls trail the score matmuls by this many m-tiles
N_CORES = 8

_NC_CACHE = None


def _build():
    import concourse.bass as bass  # noqa: F401
    from concourse import bacc, mybir
    import concourse.tile as tile

    f32 = mybir.dt.float32
    f16 = mybir.dt.float16
    bf16 = mybir.dt.bfloat16
    Exp = mybir.ActivationFunctionType.Exp

    nc = bacc.Bacc(None, target_bir_lowering=False)

    xT = nc.declare_dram_parameter("xT", [D, L], f16, isOutput=False)
    wqT = nc.declare_dram_parameter("wqT", [D, D], f16, isOutput=False)
    wkT = nc.declare_dram_parameter("wkT", [D, D], f16, isOutput=False)
    wvT = nc.declare_dram_parameter("wvT", [D, D], bf16, isOutput=False)
    woT = nc.declare_dram_parameter("woT", [D, D], f16, isOutput=False)
    mskT = nc.declare_dram_parameter("maskT", [L, L], bf16, isOutput=False)
    out = nc.declare_dram_parameter("out", [L, D], f16, isOutput=True)

    with tile.TileContext(nc) as tc:
        with (
            tc.tile_pool(name="big", bufs=1) as big,
            tc.tile_pool(name="pb", bufs=TRAIL + 1) as pbp,
            tc.tile_pool(name="sm", bufs=2) as smp,
            tc.tile_pool(name="rb", bufs=2) as rbp,
            tc.tile_pool(name="avr", bufs=2) as avrp,
            tc.tile_pool(name="yb", bufs=2) as ybp,
            tc.tile_pool(name="psmm", bufs=2, space="PSUM") as psmm,
            tc.tile_pool(name="psav", bufs=2, space="PSUM") as psav,
        ):
            x_t = [big.tile([128, L], f16, tag=f"x{i}", name=f"x{i}") for i in range(NT)]
            wq_t = [big.tile([128, D], f16, tag=f"wq{i}", name=f"wq{i}") for i in range(NT)]
            wk_t = [big.tile([128, D], f16, tag=f"wk{i}", name=f"wk{i}") for i in range(NT)]
            wv_t = [big.tile([128, D], bf16, tag=f"wv{i}", name=f"wv{i}") for i in range(NT)]
            wo_t = [big.tile([128, D], f16, tag=f"wo{i}", name=f"wo{i}") for i in range(NT)]
            mk_t = [big.tile([128, L], bf16, tag=f"mk{i}", name=f"mk{i}") for i in range(NT)]
            qt_t = [big.tile([128, L], f16, tag=f"qt{i}", name=f"qt{i}") for i in range(NT)]
            kt_t = [big.tile([128, L], f16, tag=f"kt{i}", name=f"kt{i}") for i in range(NT)]
            v_t = [big.tile([128, H, 65], bf16, tag=f"v{i}", name=f"v{i}") for i in range(NT)]
            ot_t = [big.tile([128, L], f16, tag=f"ot{i}", name=f"ot{i}") for i in range(NT)]
            ones64 = big.tile([33, 64], bf16, tag="ones64")
            negC = big.tile([128, 1], f32, tag="negC")

            # input DMAs (K/V/x first: K and V projections run first)
            for i in range(NT):
                sl = slice(i * 128, (i + 1) * 128)
                nc.sync.dma_start(out=wk_t[i][:, :], in_=wkT[sl, :])
                nc.sync.dma_start(out=x_t[i][:, :], in_=xT[sl, :])
            for i in range(NT):
                sl = slice(i * 128, (i + 1) * 128)
                nc.sync.dma_start(out=wv_t[i][:, :], in_=wvT[sl, :])
                nc.sync.dma_start(out=wq_t[i][:, :], in_=wqT[sl, :])
            for i in range(NT):
                sl = slice(i * 128, (i + 1) * 128)
                nc.sync.dma_start(out=mk_t[i][:, :], in_=mskT[sl, :])
                nc.sync.dma_start(out=wo_t[i][:, :], in_=woT[sl, :])

            nc.vector.memset(ones64[:, :], 1.0)
            nc.vector.memset(negC[:, :], EXP_SHIFT)
            for i in range(NT):
                nc.vector.memset(v_t[i][:, :, 64:65], 1.0)

            def qk_proj(w_tiles, dst, et, use_av=False):
                if use_av:
                    ps = psav.tile([128, L], f32, tag="av", name=f"projps{et}")
                else:
                    ps = psmm.tile([128, L], f32, tag="mm", name=f"projps{et}")
                for c in range(2):
                    cs = slice(c * 512, (c + 1) * 512)
                    for dt in range(NT):
                        nc.tensor.matmul(
                            ps[:, cs],
                            lhsT=w_tiles[dt][:, et * 128 : (et + 1) * 128],
                            rhs=x_t[dt][:, cs],
                            start=(dt == 0),
                            stop=(dt == NT - 1),
                        )
                nc.vector.tensor_copy(out=dst[et][:, :], in_=ps[:, :])

            def v_proj(mt):
                ps = psav.tile([128, L], f32, tag="av", name=f"vps{mt}")
                for c in range(2):
                    cs = slice(c * 512, (c + 1) * 512)
                    for dt in range(NT):
                        nc.tensor.matmul(
                            ps[:, cs],
                            lhsT=x_t[dt][:, mt * 128 : (mt + 1) * 128],
                            rhs=wv_t[dt][:, cs],
                            start=(dt == 0),
                            stop=(dt == NT - 1),
                        )
                nc.vector.tensor_copy(
                    out=v_t[mt][:, :, 0:64],
                    in_=ps[:, :].rearrange("p (h e) -> p h e", h=H),
                )

            # ---- up-front projections: all of K, Q0, then all of V
            for et in range(NT):
                qk_proj(wk_t, kt_t, et)
            qk_proj(wq_t, qt_t, 0)
            for mt in range(NT):
                v_proj(mt)

            # ---- attention: one continuous software pipeline over all
            # (head-pair, m-tile) steps. AV matmuls trail the score matmuls
            # by TRAIL steps globally (across head-pair boundaries) so the PE
            # stream never drains; K/Q projections for the next pair and the
            # deferred normalization flush slot into fixed positions.
            def flush_epilogue(p):
                pet, pavr, przb = p
                psr = psmm.tile([128, L], f32, tag="mm", name="psr")
                for j in range(2):
                    jp = slice(j * 64, (j + 1) * 64)
                    for c in range(2):
                        cs = slice(c * 512, (c + 1) * 512)
                        nc.tensor.matmul(
                            psr[jp, cs],
                            lhsT=ones64[32 * j : 32 * j + 1, :],
                            rhs=przb[32 * j : 32 * j + 1, cs],
                            start=True, stop=True,
                        )
                for j in range(2):
                    jp = slice(j * 64, (j + 1) * 64)
                    rb = rbp.tile([64, L], bf16, name="rb")
                    nc.scalar.copy(out=rb[:, :], in_=psr[jp, :])
                    nc.vector.tensor_mul(
                        ot_t[pet][jp, :], pavr[j][0:64, :], rb[:, :]
                    )

            pending = None
            av = None
            pref = {}
            navr = 0
            for g in range(NT * NT + TRAIL):
                et, mt = divmod(g, NT)
                if g < NT * NT:
                    psj = [
                        psmm.tile([128, L], f32, tag="mm", name=f"sps{j}")
                        for j in range(2)
                    ]
                    for c in range(2):
                        cs = slice(c * 512, (c + 1) * 512)
                        for j in range(2):
                            jp = slice(j * 64, (j + 1) * 64)
                            nc.tensor.matmul(
                                psj[j][:, cs],
                                lhsT=kt_t[et][jp, mt * 128 : (mt + 1) * 128],
                                rhs=qt_t[et][jp, cs],
                                start=True,
                                stop=True,
                            )
                    p2 = pbp.tile([128, 2, L], bf16, name="p2")
                    for j in range(2):
                        nc.scalar.activation(
                            p2[:, j, :], psj[j][:, :], Exp, scale=SCALE, bias=negC[:, :]
                        )
                    nc.vector.tensor_mul(
                        p2[:, :, :],
                        p2[:, :, :],
                        mk_t[mt][:, :].rearrange("p (o l) -> p o l", o=1).to_broadcast([128, 2, L]),
                    )
                    pref[(et, mt)] = p2
                    if mt == 5 and pending is not None:
                        flush_epilogue(pending)
                        pending = None
                    if mt == 2 and et + 1 < NT:
                        qk_proj(wq_t, qt_t, et + 1)
                ag = g - TRAIL
                if ag >= 0:
                    aet, amt = divmod(ag, NT)
                    if amt == 0:
                        av = [
                            psav.tile([65, L], f32, tag="av", name=f"av{j}")
                            for j in range(2)
                        ]
                    p2 = pref.pop((aet, amt))
                    for j in range(2):
                        h = 2 * aet + j
                        for c in range(2):
                            cs = slice(c * 512, (c + 1) * 512)
                            nc.tensor.matmul(
                                av[j][:, cs],
                                lhsT=v_t[amt][:, h, :],
                                rhs=p2[:, j, cs],
                                start=(amt == 0),
                                stop=(amt == NT - 1),
                            )
                    if amt == NT - 1:
                        # single bf16 eviction frees the AV psum immediately;
                        # normalization then runs lazily from SBUF
                        avrs = []
                        for j in range(2):
                            avr = avrp.tile([65, L], bf16, tag="avr", name="avr")
                            nc.vector.tensor_copy(out=avr[:, :], in_=av[j][:, :])
                            avrs.append(avr)
                        zrow = smp.tile([33, L], f32, tag="rz2", name="zrow")
                        nc.scalar.copy(out=zrow[0:1, :], in_=avrs[0][64:65, :])
                        nc.scalar.copy(out=zrow[32:33, :], in_=avrs[1][64:65, :])
                        rz = smp.tile([33, L], f32, tag="rz", name="rz")
                        nc.vector.reciprocal_approx_fast(out=rz[:, :], in_=zrow[:, :])
                        rzb = smp.tile([33, L], bf16, tag="rzb", name="rzb")
                        nc.vector.tensor_copy(out=rzb[:, :], in_=rz[:, :])
                        pending = (aet, avrs, rzb)

            if pending is not None:
                flush_epilogue(pending)
                pending = None

            # ---- output projection: y[l, f] = OT.T @ WoT
            for lt in range(NT):
                pool = psmm if lt % 2 == 0 else psav
                tag = "mm" if lt % 2 == 0 else "av"
                ps = pool.tile([128, L], f32, tag=tag, name="yps")
                for fc in range(2):
                    cs = slice(fc * 512, (fc + 1) * 512)
                    for p in range(NT):
                        nc.tensor.matmul(
                            ps[:, cs],
                            lhsT=ot_t[p][:, lt * 128 : (lt + 1) * 128],
                            rhs=wo_t[p][:, cs],
                            start=(p == 0),
                            stop=(p == NT - 1),
                        )
                y = ybp.tile([128, L], f16, name="y")
                nc.scalar.copy(out=y[:, :], in_=ps[:, :])
                nc.sync.dma_start(out=out[lt * 128 : (lt + 1) * 128, :], in_=y[:, :])

    nc.finalize()
    return nc


def _get_nc():
    global _NC_CACHE
    if _NC_CACHE is None:
        _NC_CACHE = _build()
    return _NC_CACHE


def _make_in_maps(x, mask, Wk, Wv, Wq, Wo):
    f16 = np.float16
    bf16 = ml_dtypes.bfloat16
    wqT = np.ascontiguousarray(Wq.T).astype(f16)
    wkT = np.ascontiguousarray(Wk.T).astype(f16)
    wvT = np.ascontiguousarray(Wv.T).astype(bf16)
    woT = np.ascontiguousarray(Wo.T).astype(f16)
    maskT = np.ascontiguousarray(mask[0].T).astype(bf16)
    in_maps = []
    for b in range(N_CORES):
        in_maps.append(
            {
                "xT": np.ascontiguousarray(x[b].T).astype(f16),
                "wqT": wqT,
                "wkT": wkT,
                "wvT": wvT,
                "woT": woT,
                "maskT": maskT,
            }
        )
    return in_maps


def _run(x, mask, Wk, Wv, Wq, Wo, trace=False):
    from concourse.bass_utils import run_bass_kernel_spmd

    nc = _get_nc()
    in_maps = _make_in_maps(x, mask, Wk, Wv, Wq, Wo)
    res = run_bass_kernel_spmd(nc, in_maps, list(range(N_CORES)), trace=trace)
    y = np.stack([res.results[b]["out"] for b in range(N_CORES)], axis=0)
    return y.astype(np.float32), res


def kernel(x, mask, Wk, Wv, Wq, Wo):
    y, _ = _run(x, mask, Wk, Wv, Wq, Wo, trace=False)
    return y
